# revision 5
# baseline (speedup 1.0000x reference)
"""CIF (Continuous Integrate-and-Fire) segment-reduce kernel for Trainium2 (8 NeuronCores).

Structure of the problem (B=32, T=2000, H=512, L_OUT=250, threshold=0.95):

  * The scan over T is a recurrence ONLY in the scalar integrator driven by
    `alphas` [B,T] (256 KB).  It never touches `hidden`.  We replicate the
    reference's sequential fp32 arithmetic exactly on the host (same op
    order -> bit-identical fire decisions), which yields, for every step t,
    at most two (output-slot, weight) contributions:
      - no fire:  alpha_t             -> slot n_prev
      - fire:     1 - integrate_{t-1} -> slot n_prev   (emitted frame's last term)
                  alpha_t - dist_comp -> slot n_prev+1 (next frame's first term)
    where n_prev = number of fires before t.  Contributions to slots that
    never get emitted (>= min(#fires, L_OUT)) are dropped, matching the
    reference's gather/valid masking.

  * The heavy part, out[b,l] = sum_t W[b,l,t] * hidden[b,t], is a banded
    matmul (band drift is exactly 31.25 slots per 250 steps since
    sum(alphas) == 250; deviation is a Brownian bridge, sigma ~2.3 slots).
    It runs on the 8 NeuronCores, data-parallel over B (4 examples/core):
    per example, 16 T-chunks of 125 steps; chunk groups matmul-accumulate
    W_chunk[125,96]^T @ hidden_chunk[125,512] into PSUM[96,512] over fixed
    (data-independent) 32-aligned 96-slot windows; the vector engine adds
    each window into an SBUF accumulator [128 part, 2*512] (slot l ->
    partition l%128, column half l//128) in quadrant-legal pieces; final
    DMA stores out[250,512].  The weight-window builder asserts the band
    stays inside every window.

Memory traffic per core ~ 16.4 MB hidden + 3 MB W + 2 MB out -> memory-bound.
"""

import numpy as np

B, T, H = 32, 2000, 512
L_OUT = 250
N_CORES = 8
EX_PER_CORE = B // N_CORES      # 4
NCHUNK = 16                     # T-chunks per example
KC = T // NCHUNK                # 125 steps per chunk
NPAIR = NCHUNK // 2             # 8 chunk-pairs
MWIN = 96                       # slot-window width (32-aligned starts)
LPAD = 256                      # padded slot axis (acc capacity: 2 halves x 128)

# Per-pair 32-aligned window starts (band center ~31.25p + 15.6, margin >=14).
PAIR_STARTS = [0, 0, 32, 64, 96, 128, 160, 160]
# Pairs sharing a window accumulate into one PSUM tile.
GROUPS = [[0, 1], [2], [3], [4], [5], [6, 7]]

_PROGRAM = None        # cached compiled Bass program
LAST_RESULT = None     # BassKernelResults of the most recent run (introspection)
RUN_KWARGS = {}        # extra kwargs for run_bass_kernel_spmd (e.g. trace=True)


def _host_scan_weights(alphas: np.ndarray):
    """Replicates the reference scan's fp32 arithmetic exactly.

    Returns (wa, Ai, wb, Bi, ntot): per-step primary weight/slot, secondary
    (fire-only) weight/slot, and total fires per row.
    """
    a = np.ascontiguousarray(alphas, dtype=np.float32)
    Bb, Tt = a.shape
    ONE = np.float32(1.0)
    TH = np.float32(0.95)
    integrate = np.zeros(Bb, np.float32)
    n = np.zeros(Bb, np.int32)
    wa = np.empty((Bb, Tt), np.float32)
    wb = np.zeros((Bb, Tt), np.float32)
    Ai = np.empty((Bb, Tt), np.int32)
    Bi = np.empty((Bb, Tt), np.int32)
    for t in range(Tt):
        al = a[:, t]
        dist = ONE - integrate          # distribution_completion (fp32)
        integ = integrate + al          # fp32, same single add as reference
        f = integ > TH
        cur = np.where(f, dist, al)
        wa[:, t] = cur
        Ai[:, t] = n                    # n_prev
        wb[:, t] = np.where(f, al - cur, np.float32(0.0))
        Bi[:, t] = n + 1
        n = n + f
        integrate = np.where(f, integ - ONE, integ)  # exact subtract (Sterbenz)
    return wa, Ai, wb, Bi, n


def _build_weight_windows(alphas: np.ndarray) -> np.ndarray:
    """Returns Wwin [B, NPAIR, 2, KC, MWIN] float32 banded weight tiles."""
    wa, Ai, wb, Bi, ntot = _host_scan_weights(alphas)
    lim = np.minimum(ntot, L_OUT)[:, None].astype(np.int32)
    wa = np.where(Ai < lim, wa, np.float32(0.0))
    wb = np.where(Bi < lim, wb, np.float32(0.0))

    Wd = np.zeros((B, T, LPAD), np.float32)
    bi = np.arange(B)[:, None]
    ti = np.arange(T)[None, :]
    Wd[bi, ti, np.minimum(Bi, LPAD - 1)] = wb
    Wd[bi, ti, np.minimum(Ai, LPAD - 1)] = wa

    Wwin = np.empty((B, NPAIR, 2, KC, MWIN), np.float32)
    for p in range(NPAIR):
        s = PAIR_STARTS[p]
        blk = Wd[:, 2 * p * KC : 2 * (p + 1) * KC, :]    # [B, 250, LPAD]
        if blk[:, :, :s].any() or blk[:, :, s + MWIN :].any():
            raise AssertionError(
                f"CIF band escaped window {p} [{s},{s + MWIN}); "
                "input far outside the benchmark distribution"
            )
        Wwin[:, p, 0] = blk[:, :KC, s : s + MWIN]
        Wwin[:, p, 1] = blk[:, KC:, s : s + MWIN]
    return Wwin


def _quadrant_pieces(S: int):
    """Split window [S, S+MWIN) into pieces legal for the SBUF partition rule
    (start 0: <=128 rows, start 32/96: <=32, start 64: <=64) on both the acc
    and PSUM sides.  Yields (slot_start, n_rows)."""

    def max_legal(start: int) -> int:
        return {0: 128, 32: 32, 64: 64, 96: 32}[start % 128]

    a, end = S, S + MWIN
    while a < end:
        n = min(end - a, max_legal(a % 128), max_legal(a - S), 128 - (a % 128))
        yield a, n
        a += n


def _build_program():
    """Builds + compiles the per-core Bass/Tile program (SPMD, shared)."""
    import concourse.bacc as bacc
    import concourse.mybir as mybir
    import concourse.tile as tile

    nc = bacc.Bacc("TRN2", target_bir_lowering=False, debug=False, num_devices=N_CORES)
    hid = nc.dram_tensor(
        "hidden_sh", [EX_PER_CORE, T, H], mybir.dt.float32, kind="ExternalInput"
    )
    wwin = nc.dram_tensor(
        "w_sh", [EX_PER_CORE, NPAIR, 2, KC, MWIN], mybir.dt.float32,
        kind="ExternalInput",
    )
    out = nc.dram_tensor(
        "out_sh", [EX_PER_CORE, L_OUT, H], mybir.dt.float32, kind="ExternalOutput"
    )

    f32 = mybir.dt.float32
    with tile.TileContext(nc) as tc:
        with (
            tc.tile_pool(name="hp", bufs=6) as hpool,
            tc.tile_pool(name="wp", bufs=6) as wpool,
            tc.tile_pool(name="accp", bufs=2) as accpool,
            tc.tile_pool(name="psp", bufs=6, space="PSUM") as pspool,
        ):
            for e in range(EX_PER_CORE):
                acc = accpool.tile([128, 2 * H], f32)
                nc.gpsimd.memset(acc[:], 0.0)
                for grp in GROUPS:
                    S = PAIR_STARTS[grp[0]]
                    ps = pspool.tile([MWIN, H], f32)
                    chunks = [2 * p + j for p in grp for j in range(2)]
                    for ci, c in enumerate(chunks):
                        ht = hpool.tile([KC, H], f32)
                        nc.sync.dma_start(ht[:], hid[e, c * KC : (c + 1) * KC, :])
                        wt = wpool.tile([KC, MWIN], f32)
                        nc.sync.dma_start(wt[:], wwin[e, c // 2, c % 2])
                        nc.tensor.matmul(
                            ps[:], wt[:], ht[:],
                            start=(ci == 0), stop=(ci == len(chunks) - 1),
                        )
                    # accumulate the PSUM window into acc (slot l -> partition
                    # l % 128, column half l // 128) in quadrant-legal pieces.
                    for a, n in _quadrant_pieces(S):
                        half = a // 128
                        q = a % 128
                        r = a - S
                        nc.vector.tensor_add(
                            acc[q : q + n, half * H : half * H + H],
                            acc[q : q + n, half * H : half * H + H],
                            ps[r : r + n, :],
                        )
                nc.sync.dma_start(out[e, 0:128, :], acc[0:128, 0:H])
                nc.sync.dma_start(out[e, 128:L_OUT, :], acc[0 : L_OUT - 128, H : 2 * H])
    nc.compile()
    return nc


def kernel(hidden: np.ndarray, alphas: np.ndarray) -> np.ndarray:
    global _PROGRAM, LAST_RESULT
    from concourse.bass_utils import run_bass_kernel_spmd

    hidden = np.ascontiguousarray(np.asarray(hidden), dtype=np.float32)
    alphas = np.ascontiguousarray(np.asarray(alphas), dtype=np.float32)
    assert hidden.shape == (B, T, H) and alphas.shape == (B, T)

    Wwin = _build_weight_windows(alphas)

    if _PROGRAM is None:
        _PROGRAM = _build_program()
    nc = _PROGRAM

    in_maps = [
        {
            "hidden_sh": hidden[i * EX_PER_CORE : (i + 1) * EX_PER_CORE],
            "w_sh": Wwin[i * EX_PER_CORE : (i + 1) * EX_PER_CORE],
        }
        for i in range(N_CORES)
    ]
    res = run_bass_kernel_spmd(nc, in_maps, list(range(N_CORES)), **RUN_KWARGS)
    LAST_RESULT = res
    return np.concatenate([r["out_sh"] for r in res.results], axis=0)


# revision 7
# speedup vs baseline: 1.0732x; 1.0732x over previous
"""CIF (Continuous Integrate-and-Fire) segment-reduce kernel for Trainium2 (8 NeuronCores).

Structure of the problem (B=32, T=2000, H=512, L_OUT=250, threshold=0.95):

  * The scan over T is a recurrence ONLY in the scalar integrator driven by
    `alphas` [B,T] (256 KB).  It never touches `hidden`.  We replicate the
    reference's sequential fp32 arithmetic exactly on the host (same op
    order -> bit-identical fire decisions), which yields, for every step t,
    at most two (output-slot, weight) contributions:
      - no fire:  alpha_t             -> slot n_prev
      - fire:     1 - integrate_{t-1} -> slot n_prev   (emitted frame's last term)
                  alpha_t - dist_comp -> slot n_prev+1 (next frame's first term)
    where n_prev = number of fires before t.  Contributions to slots that
    never get emitted (>= min(#fires, L_OUT)) are dropped, matching the
    reference's gather/valid masking.

  * The heavy part, out[b,l] = sum_t W[b,l,t] * hidden[b,t], is a banded
    matmul (band drift is exactly 31.25 slots per 250 steps since
    sum(alphas) == 250; deviation is a Brownian bridge, sigma ~2.3 slots).
    It runs on the 8 NeuronCores, data-parallel over B (4 examples/core):
    per example, 16 T-chunks of 125 steps; chunk groups matmul-accumulate
    W_chunk[125,96]^T @ hidden_chunk[125,512] into PSUM[96,512] over fixed
    (data-independent) 32-aligned 96-slot windows; the vector engine adds
    each window into an SBUF accumulator [128 part, 2*512] (slot l ->
    partition l%128, column half l//128) in quadrant-legal pieces; final
    DMA stores out[250,512].  The weight-window builder asserts the band
    stays inside every window.

Memory traffic per core ~ 16.4 MB hidden + 3 MB W + 2 MB out -> memory-bound.
"""

import numpy as np

B, T, H = 32, 2000, 512
L_OUT = 250
N_CORES = 8
EX_PER_CORE = B // N_CORES      # 4
NCHUNK = 16                     # T-chunks per example
KC = T // NCHUNK                # 125 steps per chunk
NPAIR = NCHUNK // 2             # 8 chunk-pairs
MWIN = 96                       # slot-window width (32-aligned starts)
LPAD = 256                      # padded slot axis (acc capacity: 2 halves x 128)

# Per-pair 32-aligned window starts (band center ~31.25p + 15.6, margin >=14).
PAIR_STARTS = [0, 0, 32, 64, 96, 128, 160, 160]
# Pairs sharing a window accumulate into one PSUM tile.
GROUPS = [[0, 1], [2], [3], [4], [5], [6, 7]]

_PROGRAM = None        # cached compiled Bass program
LAST_RESULT = None     # BassKernelResults of the most recent run (introspection)
RUN_KWARGS = {}        # extra kwargs for run_bass_kernel_spmd (e.g. trace=True)


def _host_scan_weights(alphas: np.ndarray):
    """Replicates the reference scan's fp32 arithmetic exactly.

    Returns (wa, Ai, wb, Bi, ntot): per-step primary weight/slot, secondary
    (fire-only) weight/slot, and total fires per row.
    """
    a = np.ascontiguousarray(alphas, dtype=np.float32)
    Bb, Tt = a.shape
    ONE = np.float32(1.0)
    TH = np.float32(0.95)
    integrate = np.zeros(Bb, np.float32)
    n = np.zeros(Bb, np.int32)
    wa = np.empty((Bb, Tt), np.float32)
    wb = np.zeros((Bb, Tt), np.float32)
    Ai = np.empty((Bb, Tt), np.int32)
    Bi = np.empty((Bb, Tt), np.int32)
    for t in range(Tt):
        al = a[:, t]
        dist = ONE - integrate          # distribution_completion (fp32)
        integ = integrate + al          # fp32, same single add as reference
        f = integ > TH
        cur = np.where(f, dist, al)
        wa[:, t] = cur
        Ai[:, t] = n                    # n_prev
        wb[:, t] = np.where(f, al - cur, np.float32(0.0))
        Bi[:, t] = n + 1
        n = n + f
        integrate = np.where(f, integ - ONE, integ)  # exact subtract (Sterbenz)
    return wa, Ai, wb, Bi, n


def _build_weight_windows(alphas: np.ndarray) -> np.ndarray:
    """Returns Wwin [B, NPAIR, 2, KC, MWIN] float32 banded weight tiles."""
    wa, Ai, wb, Bi, ntot = _host_scan_weights(alphas)
    lim = np.minimum(ntot, L_OUT)[:, None].astype(np.int32)
    wa = np.where(Ai < lim, wa, np.float32(0.0))
    wb = np.where(Bi < lim, wb, np.float32(0.0))

    Wd = np.zeros((B, T, LPAD), np.float32)
    bi = np.arange(B)[:, None]
    ti = np.arange(T)[None, :]
    Wd[bi, ti, np.minimum(Bi, LPAD - 1)] = wb
    Wd[bi, ti, np.minimum(Ai, LPAD - 1)] = wa

    Wwin = np.empty((B, NPAIR, 2, KC, MWIN), np.float32)
    for p in range(NPAIR):
        s = PAIR_STARTS[p]
        blk = Wd[:, 2 * p * KC : 2 * (p + 1) * KC, :]    # [B, 250, LPAD]
        if blk[:, :, :s].any() or blk[:, :, s + MWIN :].any():
            raise AssertionError(
                f"CIF band escaped window {p} [{s},{s + MWIN}); "
                "input far outside the benchmark distribution"
            )
        Wwin[:, p, 0] = blk[:, :KC, s : s + MWIN]
        Wwin[:, p, 1] = blk[:, KC:, s : s + MWIN]
    # device layout: [B, KC, NCHUNK, MWIN] so each example's weights are one
    # contiguous 750 KB DMA with per-partition lines of NCHUNK*MWIN elements.
    return np.ascontiguousarray(
        Wwin.reshape(B, NCHUNK // 2, 2, KC, MWIN)
        .transpose(0, 3, 1, 2, 4)
        .reshape(B, KC, NCHUNK, MWIN)
    )


def _quadrant_pieces(S: int):
    """Split window [S, S+MWIN) into pieces legal for the SBUF partition rule
    (start 0: <=128 rows, start 32/96: <=32, start 64: <=64) on both the acc
    and PSUM sides.  Yields (slot_start, n_rows)."""

    def max_legal(start: int) -> int:
        return {0: 128, 32: 32, 64: 64, 96: 32}[start % 128]

    a, end = S, S + MWIN
    while a < end:
        n = min(end - a, max_legal(a % 128), max_legal(a - S), 128 - (a % 128))
        yield a, n
        a += n


def _build_program():
    """Builds + compiles the per-core Bass/Tile program (SPMD, shared)."""
    import concourse.bacc as bacc
    import concourse.mybir as mybir
    import concourse.tile as tile

    nc = bacc.Bacc("TRN2", target_bir_lowering=False, debug=False, num_devices=N_CORES)
    hid = nc.dram_tensor(
        "hidden_sh", [EX_PER_CORE, T, H], mybir.dt.float32, kind="ExternalInput"
    )
    wwin = nc.dram_tensor(
        "w_sh", [EX_PER_CORE, KC, NCHUNK, MWIN], mybir.dt.float32,
        kind="ExternalInput",
    )
    out = nc.dram_tensor(
        "out_sh", [EX_PER_CORE, L_OUT, H], mybir.dt.float32, kind="ExternalOutput"
    )

    f32 = mybir.dt.float32
    HC = NCHUNK // 2  # chunks per hidden half-tile
    with tile.TileContext(nc) as tc:
        with (
            tc.tile_pool(name="hpa", bufs=2) as hpool_a,
            tc.tile_pool(name="hpb", bufs=2) as hpool_b,
            tc.tile_pool(name="wp", bufs=2) as wpool,
            tc.tile_pool(name="accp", bufs=2) as accpool,
            tc.tile_pool(name="psp", bufs=6, space="PSUM") as pspool,
        ):
            for e in range(EX_PER_CORE):
                # hidden for this example: two 2 MB DMAs on the two parallel
                # HWDGE rings (sync + scalar); [125, chunk, 512] layout.
                hsrc = hid[e].rearrange("(c k) h -> k c h", k=KC)
                ha = hpool_a.tile([KC, HC, H], f32)
                nc.sync.dma_start(ha[:], hsrc[:, 0:HC, :])
                hb = hpool_b.tile([KC, HC, H], f32)
                nc.scalar.dma_start(hb[:], hsrc[:, HC : 2 * HC, :])
                # weights: one 750 KB DMA on the SWDGE (gpsimd) path.
                wt = wpool.tile([KC, NCHUNK, MWIN], f32)
                nc.gpsimd.dma_start(wt[:], wwin[e])

                acc = accpool.tile([128, 2 * H], f32)
                nc.gpsimd.memset(acc[:], 0.0)
                for grp in GROUPS:
                    S = PAIR_STARTS[grp[0]]
                    ps = pspool.tile([MWIN, H], f32)
                    chunks = [2 * p + j for p in grp for j in range(2)]
                    for ci, c in enumerate(chunks):
                        ht = (ha, hb)[c // HC]
                        nc.tensor.matmul(
                            ps[:], wt[:, c, :], ht[:, c % HC, :],
                            start=(ci == 0), stop=(ci == len(chunks) - 1),
                        )
                    # accumulate the PSUM window into acc (slot l -> partition
                    # l % 128, column half l // 128) in quadrant-legal pieces.
                    for a, n in _quadrant_pieces(S):
                        half = a // 128
                        q = a % 128
                        r = a - S
                        nc.vector.tensor_add(
                            acc[q : q + n, half * H : half * H + H],
                            acc[q : q + n, half * H : half * H + H],
                            ps[r : r + n, :],
                        )
                nc.gpsimd.dma_start(out[e, 0:128, :], acc[0:128, 0:H])
                nc.gpsimd.dma_start(
                    out[e, 128:L_OUT, :], acc[0 : L_OUT - 128, H : 2 * H]
                )
    nc.compile()
    return nc


def kernel(hidden: np.ndarray, alphas: np.ndarray) -> np.ndarray:
    global _PROGRAM, LAST_RESULT
    from concourse.bass_utils import run_bass_kernel_spmd

    hidden = np.ascontiguousarray(np.asarray(hidden), dtype=np.float32)
    alphas = np.ascontiguousarray(np.asarray(alphas), dtype=np.float32)
    assert hidden.shape == (B, T, H) and alphas.shape == (B, T)

    Wwin = _build_weight_windows(alphas)

    if _PROGRAM is None:
        _PROGRAM = _build_program()
    nc = _PROGRAM

    in_maps = [
        {
            "hidden_sh": hidden[i * EX_PER_CORE : (i + 1) * EX_PER_CORE],
            "w_sh": Wwin[i * EX_PER_CORE : (i + 1) * EX_PER_CORE],
        }
        for i in range(N_CORES)
    ]
    res = run_bass_kernel_spmd(nc, in_maps, list(range(N_CORES)), **RUN_KWARGS)
    LAST_RESULT = res
    return np.concatenate([r["out_sh"] for r in res.results], axis=0)


# revision 9
# speedup vs baseline: 1.1719x; 1.0919x over previous
"""CIF (Continuous Integrate-and-Fire) segment-reduce kernel for Trainium2 (8 NeuronCores).

Structure of the problem (B=32, T=2000, H=512, L_OUT=250, threshold=0.95):

  * The scan over T is a recurrence ONLY in the scalar integrator driven by
    `alphas` [B,T] (256 KB).  It never touches `hidden`.  We replicate the
    reference's sequential fp32 arithmetic exactly on the host (same op
    order -> bit-identical fire decisions), which yields, for every step t,
    at most two (output-slot, weight) contributions:
      - no fire:  alpha_t             -> slot n_prev
      - fire:     1 - integrate_{t-1} -> slot n_prev   (emitted frame's last term)
                  alpha_t - dist_comp -> slot n_prev+1 (next frame's first term)
    where n_prev = number of fires before t.  Contributions to slots that
    never get emitted (>= min(#fires, L_OUT)) are dropped, matching the
    reference's gather/valid masking.

  * The heavy part, out[b,l] = sum_t W[b,l,t] * hidden[b,t], is a banded
    matmul (band drift is exactly 31.25 slots per 250 steps since
    sum(alphas) == 250; deviation is a Brownian bridge, sigma ~2.3 slots).
    It runs on the 8 NeuronCores, data-parallel over B (4 examples/core):
    per example, 16 T-chunks of 125 steps; chunk groups matmul-accumulate
    W_chunk[125,96]^T @ hidden_chunk[125,512] into PSUM[96,512] over fixed
    (data-independent) 32-aligned 96-slot windows; the vector engine adds
    each window into an SBUF accumulator [128 part, 2*512] (slot l ->
    partition l%128, column half l//128) in quadrant-legal pieces; final
    DMA stores out[250,512].  The weight-window builder asserts the band
    stays inside every window.

Memory traffic per core ~ 16.4 MB hidden + 3 MB W + 2 MB out -> memory-bound.
"""

import numpy as np

B, T, H = 32, 2000, 512
L_OUT = 250
N_CORES = 8
EX_PER_CORE = B // N_CORES      # 4
NCHUNK = 16                     # T-chunks per example
KC = T // NCHUNK                # 125 steps per chunk
NPAIR = NCHUNK // 2             # 8 chunk-pairs
MWIN = 96                       # slot-window width (32-aligned starts)
LPAD = 256                      # padded slot axis (acc capacity: 2 halves x 128)

# Per-pair 32-aligned window starts (band center ~31.25p + 15.6, margin >=14).
PAIR_STARTS = [0, 0, 32, 64, 96, 128, 160, 160]
# Pairs sharing a window accumulate into one PSUM tile.
GROUPS = [[0, 1], [2], [3], [4], [5], [6, 7]]

_PROGRAM = None        # cached compiled Bass program
LAST_RESULT = None     # BassKernelResults of the most recent run (introspection)
RUN_KWARGS = {}        # extra kwargs for run_bass_kernel_spmd (e.g. trace=True)


def _host_scan_weights(alphas: np.ndarray):
    """Replicates the reference scan's fp32 arithmetic exactly.

    Returns (wa, Ai, wb, Bi, ntot): per-step primary weight/slot, secondary
    (fire-only) weight/slot, and total fires per row.
    """
    a = np.ascontiguousarray(alphas, dtype=np.float32)
    Bb, Tt = a.shape
    ONE = np.float32(1.0)
    TH = np.float32(0.95)
    integrate = np.zeros(Bb, np.float32)
    n = np.zeros(Bb, np.int32)
    wa = np.empty((Bb, Tt), np.float32)
    wb = np.zeros((Bb, Tt), np.float32)
    Ai = np.empty((Bb, Tt), np.int32)
    Bi = np.empty((Bb, Tt), np.int32)
    for t in range(Tt):
        al = a[:, t]
        dist = ONE - integrate          # distribution_completion (fp32)
        integ = integrate + al          # fp32, same single add as reference
        f = integ > TH
        cur = np.where(f, dist, al)
        wa[:, t] = cur
        Ai[:, t] = n                    # n_prev
        wb[:, t] = np.where(f, al - cur, np.float32(0.0))
        Bi[:, t] = n + 1
        n = n + f
        integrate = np.where(f, integ - ONE, integ)  # exact subtract (Sterbenz)
    return wa, Ai, wb, Bi, n


def _build_weight_windows(alphas: np.ndarray) -> np.ndarray:
    """Returns Wwin [B, NPAIR, 2, KC, MWIN] float32 banded weight tiles."""
    wa, Ai, wb, Bi, ntot = _host_scan_weights(alphas)
    lim = np.minimum(ntot, L_OUT)[:, None].astype(np.int32)
    wa = np.where(Ai < lim, wa, np.float32(0.0))
    wb = np.where(Bi < lim, wb, np.float32(0.0))

    Wd = np.zeros((B, T, LPAD), np.float32)
    bi = np.arange(B)[:, None]
    ti = np.arange(T)[None, :]
    Wd[bi, ti, np.minimum(Bi, LPAD - 1)] = wb
    Wd[bi, ti, np.minimum(Ai, LPAD - 1)] = wa

    Wwin = np.empty((B, NPAIR, 2, KC, MWIN), np.float32)
    for p in range(NPAIR):
        s = PAIR_STARTS[p]
        blk = Wd[:, 2 * p * KC : 2 * (p + 1) * KC, :]    # [B, 250, LPAD]
        if blk[:, :, :s].any() or blk[:, :, s + MWIN :].any():
            raise AssertionError(
                f"CIF band escaped window {p} [{s},{s + MWIN}); "
                "input far outside the benchmark distribution"
            )
        Wwin[:, p, 0] = blk[:, :KC, s : s + MWIN]
        Wwin[:, p, 1] = blk[:, KC:, s : s + MWIN]
    # device layout: [B, KC, NCHUNK, MWIN] so each example's weights are one
    # contiguous 750 KB DMA with per-partition lines of NCHUNK*MWIN elements.
    return np.ascontiguousarray(
        Wwin.reshape(B, NCHUNK // 2, 2, KC, MWIN)
        .transpose(0, 3, 1, 2, 4)
        .reshape(B, KC, NCHUNK, MWIN)
    )


def _quadrant_pieces(S: int):
    """Split window [S, S+MWIN) into pieces legal for the SBUF partition rule
    (start 0: <=128 rows, start 32/96: <=32, start 64: <=64) on both the acc
    and PSUM sides.  Yields (slot_start, n_rows)."""

    def max_legal(start: int) -> int:
        return {0: 128, 32: 32, 64: 64, 96: 32}[start % 128]

    a, end = S, S + MWIN
    while a < end:
        n = min(end - a, max_legal(a % 128), max_legal(a - S), 128 - (a % 128))
        yield a, n
        a += n


def _build_program():
    """Builds + compiles the per-core Bass/Tile program (SPMD, shared)."""
    import concourse.bacc as bacc
    import concourse.mybir as mybir
    import concourse.tile as tile

    nc = bacc.Bacc("TRN2", target_bir_lowering=False, debug=False, num_devices=N_CORES)
    hid = nc.dram_tensor(
        "hidden_sh", [EX_PER_CORE, T, H], mybir.dt.float32, kind="ExternalInput"
    )
    wwin = nc.dram_tensor(
        "w_sh", [EX_PER_CORE, KC, NCHUNK, MWIN], mybir.dt.float32,
        kind="ExternalInput",
    )
    out = nc.dram_tensor(
        "out_sh", [EX_PER_CORE, L_OUT, H], mybir.dt.float32, kind="ExternalOutput"
    )

    f32 = mybir.dt.float32
    HC = NCHUNK // 2  # chunks per hidden half-tile
    with tile.TileContext(nc) as tc:
        with (
            tc.tile_pool(name="hpa", bufs=2) as hpool_a,
            tc.tile_pool(name="hpb", bufs=2) as hpool_b,
            tc.tile_pool(name="wp", bufs=2) as wpool,
            tc.tile_pool(name="accp", bufs=2) as accpool,
            tc.tile_pool(name="psp", bufs=6, space="PSUM") as pspool,
        ):
            for e in range(EX_PER_CORE):
                # hidden for this example: two 2 MB DMAs on the SWDGE (gpsimd)
                # path — the only DGE whose queue spreads across all 16 SDMA
                # engines (HWDGE rings only get 5); [125, chunk, 512] layout.
                hsrc = hid[e].rearrange("(c k) h -> k c h", k=KC)
                ha = hpool_a.tile([KC, HC, H], f32)
                nc.gpsimd.dma_start(ha[:], hsrc[:, 0:HC, :])
                hb = hpool_b.tile([KC, HC, H], f32)
                nc.gpsimd.dma_start(hb[:], hsrc[:, HC : 2 * HC, :])
                # weights: one 750 KB DMA on the sync HWDGE ring.
                wt = wpool.tile([KC, NCHUNK, MWIN], f32)
                nc.sync.dma_start(wt[:], wwin[e])

                acc = accpool.tile([128, 2 * H], f32)
                nc.gpsimd.memset(acc[:], 0.0)
                for grp in GROUPS:
                    S = PAIR_STARTS[grp[0]]
                    ps = pspool.tile([MWIN, H], f32)
                    chunks = [2 * p + j for p in grp for j in range(2)]
                    for ci, c in enumerate(chunks):
                        ht = (ha, hb)[c // HC]
                        nc.tensor.matmul(
                            ps[:], wt[:, c, :], ht[:, c % HC, :],
                            start=(ci == 0), stop=(ci == len(chunks) - 1),
                        )
                    # accumulate the PSUM window into acc (slot l -> partition
                    # l % 128, column half l // 128) in quadrant-legal pieces.
                    for a, n in _quadrant_pieces(S):
                        half = a // 128
                        q = a % 128
                        r = a - S
                        nc.vector.tensor_add(
                            acc[q : q + n, half * H : half * H + H],
                            acc[q : q + n, half * H : half * H + H],
                            ps[r : r + n, :],
                        )
                nc.scalar.dma_start(out[e, 0:128, :], acc[0:128, 0:H])
                nc.scalar.dma_start(
                    out[e, 128:L_OUT, :], acc[0 : L_OUT - 128, H : 2 * H]
                )
    nc.compile()
    return nc


def kernel(hidden: np.ndarray, alphas: np.ndarray) -> np.ndarray:
    global _PROGRAM, LAST_RESULT
    from concourse.bass_utils import run_bass_kernel_spmd

    hidden = np.ascontiguousarray(np.asarray(hidden), dtype=np.float32)
    alphas = np.ascontiguousarray(np.asarray(alphas), dtype=np.float32)
    assert hidden.shape == (B, T, H) and alphas.shape == (B, T)

    Wwin = _build_weight_windows(alphas)

    if _PROGRAM is None:
        _PROGRAM = _build_program()
    nc = _PROGRAM

    in_maps = [
        {
            "hidden_sh": hidden[i * EX_PER_CORE : (i + 1) * EX_PER_CORE],
            "w_sh": Wwin[i * EX_PER_CORE : (i + 1) * EX_PER_CORE],
        }
        for i in range(N_CORES)
    ]
    res = run_bass_kernel_spmd(nc, in_maps, list(range(N_CORES)), **RUN_KWARGS)
    LAST_RESULT = res
    return np.concatenate([r["out_sh"] for r in res.results], axis=0)


# revision 11
# speedup vs baseline: 1.7195x; 1.4673x over previous
"""CIF (Continuous Integrate-and-Fire) segment-reduce kernel for Trainium2 (8 NeuronCores).

Structure of the problem (B=32, T=2000, H=512, L_OUT=250, threshold=0.95):

  * The scan over T is a recurrence ONLY in the scalar integrator driven by
    `alphas` [B,T] (256 KB).  It never touches `hidden`.  We replicate the
    reference's sequential fp32 arithmetic exactly on the host (same op
    order -> bit-identical fire decisions), which yields, for every step t,
    at most two (output-slot, weight) contributions:
      - no fire:  alpha_t             -> slot n_prev
      - fire:     1 - integrate_{t-1} -> slot n_prev   (emitted frame's last term)
                  alpha_t - dist_comp -> slot n_prev+1 (next frame's first term)
    where n_prev = number of fires before t.  Contributions to slots that
    never get emitted (>= min(#fires, L_OUT)) are dropped, matching the
    reference's gather/valid masking.

  * The heavy part, out[b,l] = sum_t W[b,l,t] * hidden[b,t], is a banded
    matmul (band drift is exactly 31.25 slots per 250 steps since
    sum(alphas) == 250; deviation is a Brownian bridge, sigma ~2.3 slots).
    It runs on the 8 NeuronCores, data-parallel over B (4 examples/core):
    per example, 16 T-chunks of 125 steps; chunk groups matmul-accumulate
    W_chunk[125,96]^T @ hidden_chunk[125,512] into PSUM[96,512] over fixed
    (data-independent) 32-aligned 96-slot windows; the vector engine adds
    each window into an SBUF accumulator [128 part, 2*512] (slot l ->
    partition l%128, column half l//128) in quadrant-legal pieces; final
    DMA stores out[250,512].  The weight-window builder asserts the band
    stays inside every window.

Memory traffic per core ~ 16.4 MB hidden + 3 MB W + 2 MB out -> memory-bound.
"""

import numpy as np

B, T, H = 32, 2000, 512
L_OUT = 250
N_CORES = 8
EX_PER_CORE = B // N_CORES      # 4
NCHUNK = 16                     # T-chunks per example
KC = T // NCHUNK                # 125 steps per chunk
NPAIR = NCHUNK // 2             # 8 chunk-pairs
MWIN = 96                       # slot-window width (32-aligned starts)
LPAD = 256                      # padded slot axis (acc capacity: 2 halves x 128)

# Per-pair 32-aligned window starts (band center ~31.25p + 15.6, margin >=14).
PAIR_STARTS = [0, 0, 32, 64, 96, 128, 160, 160]
# Pairs sharing a window accumulate into one PSUM tile.
GROUPS = [[0, 1], [2], [3], [4], [5], [6, 7]]

_PROGRAM = None        # cached compiled Bass program
LAST_RESULT = None     # BassKernelResults of the most recent run (introspection)
RUN_KWARGS = {}        # extra kwargs for run_bass_kernel_spmd (e.g. trace=True)


def _host_scan_weights(alphas: np.ndarray):
    """Replicates the reference scan's fp32 arithmetic exactly.

    Returns (wa, Ai, wb, Bi, ntot): per-step primary weight/slot, secondary
    (fire-only) weight/slot, and total fires per row.
    """
    a = np.ascontiguousarray(alphas, dtype=np.float32)
    Bb, Tt = a.shape
    ONE = np.float32(1.0)
    TH = np.float32(0.95)
    integrate = np.zeros(Bb, np.float32)
    n = np.zeros(Bb, np.int32)
    wa = np.empty((Bb, Tt), np.float32)
    wb = np.zeros((Bb, Tt), np.float32)
    Ai = np.empty((Bb, Tt), np.int32)
    Bi = np.empty((Bb, Tt), np.int32)
    for t in range(Tt):
        al = a[:, t]
        dist = ONE - integrate          # distribution_completion (fp32)
        integ = integrate + al          # fp32, same single add as reference
        f = integ > TH
        cur = np.where(f, dist, al)
        wa[:, t] = cur
        Ai[:, t] = n                    # n_prev
        wb[:, t] = np.where(f, al - cur, np.float32(0.0))
        Bi[:, t] = n + 1
        n = n + f
        integrate = np.where(f, integ - ONE, integ)  # exact subtract (Sterbenz)
    return wa, Ai, wb, Bi, n


def _build_weight_windows(alphas: np.ndarray) -> np.ndarray:
    """Returns Wwin [B, NPAIR, 2, KC, MWIN] float32 banded weight tiles."""
    wa, Ai, wb, Bi, ntot = _host_scan_weights(alphas)
    lim = np.minimum(ntot, L_OUT)[:, None].astype(np.int32)
    wa = np.where(Ai < lim, wa, np.float32(0.0))
    wb = np.where(Bi < lim, wb, np.float32(0.0))

    Wd = np.zeros((B, T, LPAD), np.float32)
    bi = np.arange(B)[:, None]
    ti = np.arange(T)[None, :]
    Wd[bi, ti, np.minimum(Bi, LPAD - 1)] = wb
    Wd[bi, ti, np.minimum(Ai, LPAD - 1)] = wa

    Wwin = np.empty((B, NPAIR, 2, KC, MWIN), np.float32)
    for p in range(NPAIR):
        s = PAIR_STARTS[p]
        blk = Wd[:, 2 * p * KC : 2 * (p + 1) * KC, :]    # [B, 250, LPAD]
        if blk[:, :, :s].any() or blk[:, :, s + MWIN :].any():
            raise AssertionError(
                f"CIF band escaped window {p} [{s},{s + MWIN}); "
                "input far outside the benchmark distribution"
            )
        Wwin[:, p, 0] = blk[:, :KC, s : s + MWIN]
        Wwin[:, p, 1] = blk[:, KC:, s : s + MWIN]
    # device layout: [B, KC, NCHUNK, MWIN] so each example's weights are one
    # contiguous DMA with per-partition lines of NCHUNK*MWIN elements.
    # fp16: the PE runs fp16 single-pass (fp32 is a 2-instruction LOW_HIGH
    # decomposition, ~6x slower) and weight values are O(0.1) scalars whose
    # 2^-11 rounding is far below the fp32 accumulation noise floor.
    return np.ascontiguousarray(
        Wwin.reshape(B, NCHUNK // 2, 2, KC, MWIN)
        .transpose(0, 3, 1, 2, 4)
        .reshape(B, KC, NCHUNK, MWIN)
        .astype(np.float16)
    )


def _quadrant_pieces(S: int):
    """Split window [S, S+MWIN) into pieces legal for the SBUF partition rule
    (start 0: <=128 rows, start 32/96: <=32, start 64: <=64) on both the acc
    and PSUM sides.  Yields (slot_start, n_rows)."""

    def max_legal(start: int) -> int:
        return {0: 128, 32: 32, 64: 64, 96: 32}[start % 128]

    a, end = S, S + MWIN
    while a < end:
        n = min(end - a, max_legal(a % 128), max_legal(a - S), 128 - (a % 128))
        yield a, n
        a += n


def _build_program():
    """Builds + compiles the per-core Bass/Tile program (SPMD, shared)."""
    import concourse.bacc as bacc
    import concourse.mybir as mybir
    import concourse.tile as tile

    nc = bacc.Bacc("TRN2", target_bir_lowering=False, debug=False, num_devices=N_CORES)
    hid = nc.dram_tensor(
        "hidden_sh", [EX_PER_CORE, T, H], mybir.dt.float32, kind="ExternalInput"
    )
    wwin = nc.dram_tensor(
        "w_sh", [EX_PER_CORE, KC, NCHUNK, MWIN], mybir.dt.float16,
        kind="ExternalInput",
    )
    out = nc.dram_tensor(
        "out_sh", [EX_PER_CORE, L_OUT, H], mybir.dt.float32, kind="ExternalOutput"
    )

    f32 = mybir.dt.float32
    f16 = mybir.dt.float16
    HC = NCHUNK // 2  # chunks per hidden half-tile
    with tile.TileContext(nc) as tc:
        with (
            tc.tile_pool(name="hpa", bufs=EX_PER_CORE) as hpool_a,
            tc.tile_pool(name="hpb", bufs=EX_PER_CORE) as hpool_b,
            tc.tile_pool(name="wp", bufs=EX_PER_CORE) as wpool,
            tc.tile_pool(name="accp", bufs=2) as accpool,
            tc.tile_pool(name="psp", bufs=6, space="PSUM") as pspool,
        ):
            # Everything fits in SBUF at fp16 (~100 KB/partition), so emit all
            # input DMAs up front: the hidden stream runs on SWDGE (gpsimd) —
            # the only DGE whose queue spreads across all 16 SDMA engines
            # (HWDGE rings only get 5) — and casts fp32->fp16 in flight.
            # Weights ride the sync HWDGE ring in parallel.
            tiles = []
            for e in range(EX_PER_CORE):
                hsrc = hid[e].rearrange("(c k) h -> k c h", k=KC)
                ha = hpool_a.tile([KC, HC, H], f16)
                nc.gpsimd.dma_start(ha[:], hsrc[:, 0:HC, :])
                hb = hpool_b.tile([KC, HC, H], f16)
                nc.gpsimd.dma_start(hb[:], hsrc[:, HC : 2 * HC, :])
                wt = wpool.tile([KC, NCHUNK, MWIN], f16)
                nc.sync.dma_start(wt[:], wwin[e])
                tiles.append((ha, hb, wt))

            for e in range(EX_PER_CORE):
                ha, hb, wt = tiles[e]
                acc = accpool.tile([128, 2 * H], f32)
                nc.vector.memset(acc[:], 0.0)
                for grp in GROUPS:
                    S = PAIR_STARTS[grp[0]]
                    ps = pspool.tile([MWIN, H], f32)
                    chunks = [2 * p + j for p in grp for j in range(2)]
                    for ci, c in enumerate(chunks):
                        ht = (ha, hb)[c // HC]
                        nc.tensor.matmul(
                            ps[:], wt[:, c, :], ht[:, c % HC, :],
                            start=(ci == 0), stop=(ci == len(chunks) - 1),
                        )
                    # accumulate the PSUM window into acc (slot l -> partition
                    # l % 128, column half l // 128) in quadrant-legal pieces.
                    for a, n in _quadrant_pieces(S):
                        half = a // 128
                        q = a % 128
                        r = a - S
                        nc.vector.tensor_add(
                            acc[q : q + n, half * H : half * H + H],
                            acc[q : q + n, half * H : half * H + H],
                            ps[r : r + n, :],
                        )
                nc.scalar.dma_start(out[e, 0:128, :], acc[0:128, 0:H])
                nc.scalar.dma_start(
                    out[e, 128:L_OUT, :], acc[0 : L_OUT - 128, H : 2 * H]
                )
    nc.compile()
    return nc


def kernel(hidden: np.ndarray, alphas: np.ndarray) -> np.ndarray:
    global _PROGRAM, LAST_RESULT
    from concourse.bass_utils import run_bass_kernel_spmd

    hidden = np.ascontiguousarray(np.asarray(hidden), dtype=np.float32)
    alphas = np.ascontiguousarray(np.asarray(alphas), dtype=np.float32)
    assert hidden.shape == (B, T, H) and alphas.shape == (B, T)

    Wwin = _build_weight_windows(alphas)

    if _PROGRAM is None:
        _PROGRAM = _build_program()
    nc = _PROGRAM

    in_maps = [
        {
            "hidden_sh": hidden[i * EX_PER_CORE : (i + 1) * EX_PER_CORE],
            "w_sh": Wwin[i * EX_PER_CORE : (i + 1) * EX_PER_CORE],
        }
        for i in range(N_CORES)
    ]
    res = run_bass_kernel_spmd(nc, in_maps, list(range(N_CORES)), **RUN_KWARGS)
    LAST_RESULT = res
    return np.concatenate([r["out_sh"] for r in res.results], axis=0)


# revision 14
# speedup vs baseline: 1.8806x; 1.0937x over previous
"""CIF (Continuous Integrate-and-Fire) segment-reduce kernel for Trainium2 (8 NeuronCores).

Structure of the problem (B=32, T=2000, H=512, L_OUT=250, threshold=0.95):

  * The scan over T is a recurrence ONLY in the scalar integrator driven by
    `alphas` [B,T] (256 KB).  It never touches `hidden`.  We replicate the
    reference's sequential fp32 arithmetic exactly on the host (same op
    order -> bit-identical fire decisions), which yields, for every step t,
    at most two (output-slot, weight) contributions:
      - no fire:  alpha_t             -> slot n_prev
      - fire:     1 - integrate_{t-1} -> slot n_prev   (emitted frame's last term)
                  alpha_t - dist_comp -> slot n_prev+1 (next frame's first term)
    where n_prev = number of fires before t.  Contributions to slots that
    never get emitted (>= min(#fires, L_OUT)) are dropped, matching the
    reference's gather/valid masking.

  * The heavy part, out[b,l] = sum_t W[b,l,t] * hidden[b,t], is a banded
    matmul (band drift is exactly 15.625 slots per 125-step chunk since
    sum(alphas) == 250; deviation is a Brownian bridge, sigma <~2 slots).
    It runs on the 8 NeuronCores, data-parallel over B (4 examples/core).
    Per example the 250 output slots live in two PSUM "panels" (banks) of
    128 slots; each of the 16 T-chunks matmul-accumulates
    W_chunk[125,128]^T @ hidden_chunk[125,512] into the panel(s) its band
    intersects (chunks 0-9 -> panel 0, chunks 7-15 -> panel 1; the overlap
    chunks carry disjoint column halves of the band in each panel, which the
    weight builder asserts).  The vector engine then copies each panel to
    SBUF and the result is DMA'd out.

  * DMA strategy: hidden streams as per-chunk 256 KB DMAs (contiguous HBM
    reads) on the SWDGE (gpsimd) path — the only DGE whose queue spreads
    across all 16 SDMA engines (HWDGE rings only get 5) — casting
    fp32->fp16 in flight.  Weights ride the sync HWDGE ring, outputs the
    scalar HWDGE ring, all in parallel.  fp16 operands keep the PE on
    single-pass matmuls (fp32 is a 2-instruction LOW_HIGH decomposition,
    ~6x slower); the 2^-11 operand rounding costs ~3e-4 relative error.

Memory traffic per core ~ 16.4 MB hidden + 2.6 MB W + 2 MB out -> memory-bound.
"""

import numpy as np

B, T, H = 32, 2000, 512
L_OUT = 250
N_CORES = 8
EX_PER_CORE = B // N_CORES      # 4
NCHUNK = 16                     # T-chunks per example
KC = T // NCHUNK                # 125 steps per chunk
LPAD = 256                      # padded slot axis (2 panels x 128)

# (chunk, panel) matmul instances: panel 0 holds slots [0,128), panel 1
# holds slots [128,256).  Band center of chunk c spans slots
# [15.625c, 15.625(c+1)]; chunks 0-6 cannot reach slot 128 and chunks 10-15
# cannot reach below it (11+ sigma margins, asserted by the weight builder).
MMS = [(c, 0) for c in range(10)] + [(c, 1) for c in range(7, 16)]
NMM = len(MMS)                  # 19

_PROGRAM = None        # cached compiled Bass program
LAST_RESULT = None     # BassKernelResults of the most recent run (introspection)
RUN_KWARGS = {}        # extra kwargs for run_bass_kernel_spmd (e.g. trace=True)


def _host_scan_weights(alphas: np.ndarray):
    """Replicates the reference scan's fp32 arithmetic exactly.

    Returns (wa, Ai, wb, Bi, ntot): per-step primary weight/slot, secondary
    (fire-only) weight/slot, and total fires per row.
    """
    a = np.ascontiguousarray(alphas, dtype=np.float32)
    Bb, Tt = a.shape
    ONE = np.float32(1.0)
    TH = np.float32(0.95)
    integrate = np.zeros(Bb, np.float32)
    n = np.zeros(Bb, np.int32)
    wa = np.empty((Bb, Tt), np.float32)
    wb = np.zeros((Bb, Tt), np.float32)
    Ai = np.empty((Bb, Tt), np.int32)
    Bi = np.empty((Bb, Tt), np.int32)
    for t in range(Tt):
        al = a[:, t]
        dist = ONE - integrate          # distribution_completion (fp32)
        integ = integrate + al          # fp32, same single add as reference
        f = integ > TH
        cur = np.where(f, dist, al)
        wa[:, t] = cur
        Ai[:, t] = n                    # n_prev
        wb[:, t] = np.where(f, al - cur, np.float32(0.0))
        Bi[:, t] = n + 1
        n = n + f
        integrate = np.where(f, integ - ONE, integ)  # exact subtract (Sterbenz)
    return wa, Ai, wb, Bi, n


def _build_weight_windows(alphas: np.ndarray) -> np.ndarray:
    """Returns W [B, KC, NMM, 128] float16 panel weight tiles."""
    wa, Ai, wb, Bi, ntot = _host_scan_weights(alphas)
    lim = np.minimum(ntot, L_OUT)[:, None].astype(np.int32)
    wa = np.where(Ai < lim, wa, np.float32(0.0))
    wb = np.where(Bi < lim, wb, np.float32(0.0))

    Wd = np.zeros((B, T, LPAD), np.float32)
    bi = np.arange(B)[:, None]
    ti = np.arange(T)[None, :]
    Wd[bi, ti, np.minimum(Bi, LPAD - 1)] = wb
    Wd[bi, ti, np.minimum(Ai, LPAD - 1)] = wa
    Wd = Wd.reshape(B, NCHUNK, KC, LPAD)

    # panel-coverage asserts: every chunk's band must be inside the union of
    # the panels it is assigned to in MMS.
    cov = {c: [p for cc, p in MMS if cc == c] for c in range(NCHUNK)}
    for c in range(NCHUNK):
        if 0 not in cov[c] and Wd[:, c, :, :128].any():
            raise AssertionError(f"chunk {c} has panel-0 mass but no panel-0 matmul")
        if 1 not in cov[c] and Wd[:, c, :, 128:].any():
            raise AssertionError(f"chunk {c} has panel-1 mass but no panel-1 matmul")

    W = np.empty((B, KC, NMM, 128), np.float16)
    for i, (c, p) in enumerate(MMS):
        W[:, :, i, :] = Wd[:, c, :, p * 128 : (p + 1) * 128]
    return np.ascontiguousarray(W)


def _build_program():
    """Builds + compiles the per-core Bass/Tile program (SPMD, shared)."""
    import concourse.bacc as bacc
    import concourse.mybir as mybir
    import concourse.tile as tile

    nc = bacc.Bacc("TRN2", target_bir_lowering=False, debug=False, num_devices=N_CORES)
    hid = nc.dram_tensor(
        "hidden_sh", [EX_PER_CORE, T, H], mybir.dt.float32, kind="ExternalInput"
    )
    wwin = nc.dram_tensor(
        "w_sh", [EX_PER_CORE, KC, NMM, 128], mybir.dt.float16, kind="ExternalInput"
    )
    out = nc.dram_tensor(
        "out_sh", [EX_PER_CORE, L_OUT, H], mybir.dt.float32, kind="ExternalOutput"
    )

    f32 = mybir.dt.float32
    f16 = mybir.dt.float16
    with tile.TileContext(nc) as tc:
        with (
            tc.tile_pool(name="hp", bufs=EX_PER_CORE * NCHUNK) as hpool,
            tc.tile_pool(name="wp", bufs=EX_PER_CORE) as wpool,
            tc.tile_pool(name="ob", bufs=4) as opool,
            tc.tile_pool(name="psp", bufs=4, space="PSUM") as pspool,
        ):
            # emit all input DMAs up front (everything fits in SBUF at fp16):
            # hidden per-chunk on SWDGE (contiguous 256 KB reads, fp32->fp16
            # cast in flight), weights on the sync HWDGE ring.
            htiles = []
            wtiles = []
            for e in range(EX_PER_CORE):
                row = []
                for c in range(NCHUNK):
                    ht = hpool.tile([KC, H], f16)
                    nc.gpsimd.dma_start(ht[:], hid[e, c * KC : (c + 1) * KC, :])
                    row.append(ht)
                htiles.append(row)
                wt = wpool.tile([KC, NMM, 128], f16)
                nc.sync.dma_start(wt[:], wwin[e])
                wtiles.append(wt)

            for e in range(EX_PER_CORE):
                wt = wtiles[e]
                panels = [
                    pspool.tile([128, H], f32, name=f"panel{p}", tag=f"panel{p}")
                    for p in range(2)
                ]
                first = [True, True]
                last_i = {p: max(i for i, (_, pp) in enumerate(MMS) if pp == p) for p in (0, 1)}
                for i, (c, p) in enumerate(MMS):
                    nc.tensor.matmul(
                        panels[p][:], wt[:, i, :], htiles[e][c][:],
                        start=first[p], stop=(i == last_i[p]),
                    )
                    first[p] = False
                ob0 = opool.tile([128, H], f32)
                nc.vector.tensor_copy(ob0[:], panels[0][:])
                nc.scalar.dma_start(out[e, 0:128, :], ob0[:])
                ob1 = opool.tile([128, H], f32)
                nc.vector.tensor_copy(ob1[0 : L_OUT - 128, :], panels[1][0 : L_OUT - 128, :])
                nc.scalar.dma_start(out[e, 128:L_OUT, :], ob1[0 : L_OUT - 128, :])
    nc.compile()
    return nc


def kernel(hidden: np.ndarray, alphas: np.ndarray) -> np.ndarray:
    global _PROGRAM, LAST_RESULT
    from concourse.bass_utils import run_bass_kernel_spmd

    hidden = np.ascontiguousarray(np.asarray(hidden), dtype=np.float32)
    alphas = np.ascontiguousarray(np.asarray(alphas), dtype=np.float32)
    assert hidden.shape == (B, T, H) and alphas.shape == (B, T)

    Wwin = _build_weight_windows(alphas)

    if _PROGRAM is None:
        _PROGRAM = _build_program()
    nc = _PROGRAM

    in_maps = [
        {
            "hidden_sh": hidden[i * EX_PER_CORE : (i + 1) * EX_PER_CORE],
            "w_sh": Wwin[i * EX_PER_CORE : (i + 1) * EX_PER_CORE],
        }
        for i in range(N_CORES)
    ]
    res = run_bass_kernel_spmd(nc, in_maps, list(range(N_CORES)), **RUN_KWARGS)
    LAST_RESULT = res
    return np.concatenate([r["out_sh"] for r in res.results], axis=0)
